# revision 34
# baseline (speedup 1.0000x reference)
"""GQA (B=1, S=2048, D=4096, H=32, G=8) on 8 TRN2 NeuronCores.

Tensor-parallel over heads: core c owns query heads 4c..4c+3 and KV group c.
v2: all-bf16 datapath (weights/x/k/q/probs/ctx/Wo and the AllGather wire),
SCH=512 sequence chunks, per-head-pair AllGathers issued mid-chunk, Wo
matmuls interleaved into the attention chunks as tensor-engine filler, and
output produced transposed ([OC, S]) so Wo keeps its weights stationary.
Host slices weights / transposes x / re-transposes the output.

Self-contained: no sibling imports; hardcoded shapes.
"""
import contextlib
import ctypes
import math
import os
import sys
import types

import ml_dtypes
import numpy as np

os.environ.setdefault("MYCRO_LOCAL_CACHE", "1")

for _p in ("/opt/trn_rl_repo", "/root/.axon_site/_ro/trn_rl_repo"):
    if _p not in sys.path and os.path.isdir(_p):
        sys.path.append(_p)

import concourse.bass as bass
import concourse.tile as tile
from concourse import mybir
from concourse.bass_utils import run_bass_kernel_spmd
from concourse.masks import make_identity

# ---------------------------------------------------------------- profiling shim
_SO_PATH = "/opt/axon/libaxon_pjrt.so"
_hook_holder = [None]


def _ntff_profile_via_ctypes(so_path):
    try:
        lib = ctypes.CDLL(so_path)
    except OSError:
        return None
    if not hasattr(lib, "axon_start_nrt_profile"):
        return None
    lib.axon_start_nrt_profile.argtypes = [
        ctypes.POINTER(ctypes.c_int64),
        ctypes.c_size_t,
    ]
    lib.axon_start_nrt_profile.restype = ctypes.c_int64
    lib.axon_stop_nrt_profile.argtypes = [ctypes.c_char_p]
    lib.axon_stop_nrt_profile.restype = ctypes.c_int64

    @contextlib.contextmanager
    def _hook(output_dir, device_ids):
        import jax

        jax.devices()
        if device_ids:
            ids = (ctypes.c_int64 * len(device_ids))(*device_ids)
            rc = lib.axon_start_nrt_profile(ids, len(device_ids))
        else:
            rc = lib.axon_start_nrt_profile(None, 0)
        if rc != 0:
            raise RuntimeError(f"axon_start_nrt_profile rc={rc}")
        try:
            yield
        finally:
            n = lib.axon_stop_nrt_profile(str(output_dir).encode())
            if n <= 0:
                print(f"WARNING: ntff capture wrote {n} files", file=sys.stderr)

    return _hook


def _install_prof_shim():
    if "antenv.axon_hooks" not in sys.modules:
        mod = types.ModuleType("antenv.axon_hooks")
        mod.set_axon_ntff_profile_hook = lambda h: _hook_holder.__setitem__(0, h)
        mod.get_axon_ntff_profile_hook = lambda: _hook_holder[0]
        sys.modules["antenv.axon_hooks"] = mod
    _hook_holder[0] = _ntff_profile_via_ctypes(_SO_PATH)
    import concourse.bass_utils as bu

    bu.upload_artifacts = lambda tmpdir: tmpdir


_install_prof_shim()

# ------------------------------------------------------------- wait-split pass
def _split_multi_waits(nc, maxw=1):
    """walrus in this container allows only one sync-wait per instruction;
    split extras onto nops inserted before the offender (same engine/block)."""

    def _remove_by_name(name):
        for f in nc.m.functions:
            for bb in f.blocks:
                for i, inst in enumerate(bb.instructions):
                    if inst.name == name:
                        lst = bb.instructions
                        del lst[i]
                        bb.instructions = lst
                        return inst
        raise KeyError(name)

    offenders = []
    for f in nc.m.functions:
        for bb in f.blocks:
            for inst in bb.instructions:
                si = inst.sync_info
                if si and si.on_wait and len(si.on_wait) > maxw:
                    offenders.append(inst.name)
    for name in offenders:
        target = None
        for f in nc.m.functions:
            for bb in f.blocks:
                for idx, inst in enumerate(bb.instructions):
                    if inst.name == name:
                        target = (bb, inst)
                        break
                if target:
                    break
            if target:
                break
        bb, inst = target
        waits = list(inst.sync_info.on_wait)
        updates = list(inst.sync_info.on_update or [])
        chunks = [waits[i:i + maxw] for i in range(0, len(waits), maxw)]
        nops = []
        for ch in chunks[:-1]:
            bnop = nc.engines[inst.engine].nop(nofuse=True, hint="waitsplit")
            nop_inst = _remove_by_name(bnop.ins.name)
            nop_inst.sync_info = mybir.SyncInfo(on_wait=ch, on_update=[])
            nops.append(nop_inst)
        inst.sync_info = mybir.SyncInfo(on_wait=chunks[-1], on_update=updates)
        lst = bb.instructions
        idx = next(i for i, x in enumerate(lst) if x.name == name)
        lst[idx:idx] = nops
        bb.instructions = lst
    return len(offenders)


# ------------------------------------------------------------------- constants
B, S, D = 1, 2048, 4096
H, G = 32, 8
HD = D // H            # 128
NC = 8                 # cores
HPC = H // NC          # q heads per core = 4
OC = D // NC           # out columns per core = 512
P = 128
KT = D // P            # 32 contraction tiles over D
SCH = 512              # sequence chunk width
NSC = S // SCH         # 4
NKB = S // P           # 16 key tiles of 128
NQS = SCH // P         # 4 q-subchunks per chunk
SCALE = float(1.0 / math.sqrt(float(HD)))

f32 = mybir.dt.float32
bf16 = mybir.dt.bfloat16

Copy = mybir.ActivationFunctionType.Copy
Exp = mybir.ActivationFunctionType.Exp


class _WoSched:
    """FIFO of deferred Wo closures with release markers: a ("wait", key)
    entry blocks consumption until release(key) — used to hold units back
    until their AllGather has had time to complete."""

    def __init__(self):
        self.units = []
        self.released = set()

    def add(self, fn):
        self.units.append(fn)

    def add_marker(self, key):
        self.units.append(("wait", key))

    def release(self, key):
        self.released.add(key)

    def take(self, n):
        while n > 0 and self.units:
            head = self.units[0]
            if isinstance(head, tuple):
                if head[1] in self.released:
                    self.units.pop(0)
                    continue
                return
            self.units.pop(0)()
            n -= 1

    def drain(self):
        self.take(len(self.units) + 8)

    def force_drain(self):
        for u in self.units:
            if not isinstance(u, tuple):
                u()
        self.units = []


def _build_program():
    nc = bass.Bass()
    xT = nc.declare_dram_parameter("xT", [P, NSC, KT, SCH], bf16, isOutput=False)
    wq = nc.declare_dram_parameter("wq", [P, HPC, KT, HD], bf16, isOutput=False)
    wk = nc.declare_dram_parameter("wk", [P, KT, HD], bf16, isOutput=False)
    wv = nc.declare_dram_parameter("wv", [P, KT, HD], bf16, isOutput=False)
    wo = nc.declare_dram_parameter("wo", [P, KT, OC], bf16, isOutput=False)
    cosT = nc.declare_dram_parameter("cosT", [HD, S], bf16, isOutput=False)
    sinT = nc.declare_dram_parameter("sinT", [HD, S], bf16, isOutput=False)
    tri = nc.declare_dram_parameter("tri", [P, P], bf16, isOutput=False)
    out = nc.declare_dram_parameter("out", [OC, S], bf16, isOutput=True)

    # one AllGather per (chunk, head-pair): in [2*P, SCH], out [NC*2*P, SCH]
    cc_ins = [[nc.dram_tensor(f"cc_in{sc}_{pr}", [2 * P, SCH], bf16)
               for pr in range(2)] for sc in range(NSC)]
    cc_outs = [[nc.dram_tensor(f"cc_out{sc}_{pr}", [NC * 2 * P, SCH], bf16,
                               addr_space="Shared")
                for pr in range(2)] for sc in range(NSC)]
    bar_in = nc.dram_tensor("bar_in", [P, 8], bf16)
    bar_out = nc.dram_tensor("bar_out", [NC * P, 8], bf16,
                             addr_space="Shared")

    # global 128-row D-block kt = 4*rank + head -> (pair, tile within cc_out)
    def kt_loc(kt):
        r, h = kt // HPC, kt % HPC
        return h // 2, r * 2 + (h % 2)

    with tile.TileContext(nc) as tc:
        with (
            tc.tile_pool(name="singles", bufs=1) as singles,
            tc.tile_pool(name="xp", bufs=6) as xpp,
            tc.tile_pool(name="qts", bufs=6) as qtsp,
            tc.tile_pool(name="pt", bufs=20) as ptp,
            tc.tile_pool(name="work", bufs=4) as work,
            tc.tile_pool(name="stg", bufs=3) as stgp,
            tc.tile_pool(name="ccp", bufs=12) as ccp,
            tc.tile_pool(name="osb", bufs=3) as osbp,
            tc.tile_pool(name="ps", bufs=1, space="PSUM") as psp,
        ):
            # ---- constants loaded in compute-dependency order
            # rank-alignment barrier: fires immediately on the CC ring so the
            # startup skew is absorbed while weights stream in
            nc.gpsimd.collective_compute(
                "AllGather", mybir.AluOpType.bypass,
                replica_groups=[list(range(NC))],
                ins=[bar_in[:]], outs=[bar_out[:]])

            identf = singles.tile([P, P], f32)
            make_identity(nc, identf[:])
            ident = singles.tile([P, P], bf16)
            nc.vector.tensor_copy(ident[:], identf[:])
            trim = singles.tile([P, P], bf16)
            nc.sync.dma_start(out=trim[:], in_=tri[:])

            def issue_x(sc):
                tiles = []
                for g in range(KT // 8):
                    t = xpp.tile([P, 8, SCH], bf16, tag="xp", bufs=6)
                    nc.sync.dma_start(out=t[:], in_=xT[:, sc, 8 * g:8 * g + 8, :])
                    tiles.append(t)
                return tiles

            # stagger wk pieces with x pieces so the first projections can
            # begin as soon as ~0.5 MB has landed
            wk_sb = singles.tile([P, KT, HD], bf16)
            wv_sb = singles.tile([P, KT, HD], bf16)
            xtg_cur = []
            for g in range(KT // 8):
                nc.sync.dma_start(out=wk_sb[:, 8 * g:8 * g + 8, :],
                                  in_=wk[:, 8 * g:8 * g + 8, :])
                t = xpp.tile([P, 8, SCH], bf16, tag="xp", bufs=6)
                nc.sync.dma_start(out=t[:], in_=xT[:, 0, 8 * g:8 * g + 8, :])
                xtg_cur.append(t)
            nc.sync.dma_start(out=wv_sb[:], in_=wv[:])

            wq_sb = singles.tile([P, HPC, KT, HD], bf16)
            nc.sync.dma_start(out=wq_sb[:, 0], in_=wq[:, 0])
            cos_sb = singles.tile([HD, S], bf16)
            nc.sync.dma_start(out=cos_sb[:], in_=cosT[:])
            sin_sb = singles.tile([HD, S], bf16)
            nc.sync.dma_start(out=sin_sb[:], in_=sinT[:])
            for h in range(1, HPC):
                nc.sync.dma_start(out=wq_sb[:, h], in_=wq[:, h])
            wo_sb = singles.tile([P, KT, OC], bf16)
            nc.sync.dma_start(out=wo_sb[:], in_=wo[:])

            kT_all = singles.tile([HD, S], bf16)
            vp_all = singles.tile([P, NKB, HD + 1], bf16)
            nc.vector.memset(vp_all[:], 1.0)

            wos = _WoSched()

            # ---------- Wo split `spl`: two ob-pair passes over all 32 kt
            def queue_wo_split(spl):
                kts = sorted(range(KT), key=lambda kt: kt_loc(kt))
                wos.add_marker((spl + 1, 3 if spl == 0 else 0))
                for pss in (0, 1):
                    obs = (2 * pss, 2 * pss + 1)
                    accs = {}
                    ccts = {}
                    PF = 6

                    def issue_dma(i, kts=kts, ccts=ccts):
                        pr, t = kt_loc(kts[i])
                        cct = ccp.tile([P, SCH], bf16, tag="ccp")
                        nc.sync.dma_start(
                            out=cct[:],
                            in_=cc_outs[spl][pr][:].rearrange(
                                "(t p) s -> p t s", p=P)[:, t, :])
                        ccts[i] = cct

                    def emit(i, pss=pss, obs=obs, kts=kts, accs=accs,
                             ccts=ccts, PF=PF, issue_dma=issue_dma):
                        if i == 0:
                            for ob in obs:
                                accs[ob] = psp.tile([P, SCH], f32, tag="o",
                                                    bufs=2, name=f"wo{spl}_{ob}")
                            for j in range(min(PF, len(kts))):
                                issue_dma(j)
                        if i + PF < len(kts):
                            issue_dma(i + PF)
                        cct = ccts.pop(i)
                        first, last = i == 0, i == len(kts) - 1
                        for ob in obs:
                            nc.tensor.matmul(
                                accs[ob][:], wo_sb[:, kts[i], ob * P:(ob + 1) * P],
                                cct[:], start=first, stop=last)
                        if last:
                            for ob in obs:
                                ps_o = accs.pop(ob)
                                o_sb = osbp.tile([P, SCH], bf16, tag="osb")
                                nc.vector.tensor_copy(o_sb[:], ps_o[:])
                                nc.scalar.dma_start(
                                    out=out[ob * P:(ob + 1) * P,
                                            spl * SCH:(spl + 1) * SCH],
                                    in_=o_sb[:])

                    for i in range(len(kts)):
                        if pss == 0 and i > 0 and kt_loc(kts[i])[0] == 1 \
                                and kt_loc(kts[i - 1])[0] == 0:
                            wos.add_marker((spl + 1, 3))
                        wos.add(lambda i=i, emit=emit: emit(i))

            def rope_evict(ps_t, dst, tab0):
                """ps_t: PSUM [HD, SCH] pre-rope; writes dst [HD, SCH] bf16
                using rope tables at absolute position tab0."""
                rot = work.tile([HD, SCH], f32, tag="rot", bufs=2)
                nc.vector.tensor_scalar_mul(rot[0:64, :], ps_t[64:128, :], -1.0)
                nc.vector.tensor_copy(rot[64:128, :], ps_t[0:64, :])
                m1 = work.tile([HD, SCH], f32, tag="m1", bufs=2)
                nc.vector.tensor_mul(m1[:], ps_t[:], cos_sb[:, tab0:tab0 + SCH])
                nc.vector.tensor_mul(rot[:], rot[:], sin_sb[:, tab0:tab0 + SCH])
                nc.vector.tensor_add(dst, m1[:], rot[:])

            # ---------- projection work for chunk sc as filler closures;
            # dispatched inside the PREVIOUS chunk's attention
            def make_proj_work(sc, xtg):
                s0 = sc * SCH
                st = {"qts": []}

                def xts(kt, xtg=xtg):
                    return xtg[kt // 8][:, kt % 8, :]

                vcs = {}

                def g_k(sc=sc, s0=s0, xts=xts):
                    ps_k = psp.tile([P, SCH], f32, tag="a", bufs=3,
                                    name=f"psk{sc}")
                    for kt in range(KT):
                        nc.tensor.matmul(ps_k[:], wk_sb[:, kt, :], xts(kt),
                                         start=(kt == 0), stop=(kt == KT - 1))
                    rope_evict(ps_k, kT_all[:, s0:s0 + SCH], s0)

                def g_v(sc=sc, xts=xts, vcs=vcs):
                    ps_v = psp.tile([P, SCH], f32, tag="a", bufs=3,
                                    name=f"psv{sc}")
                    for kt in range(KT):
                        nc.tensor.matmul(ps_v[:], wv_sb[:, kt, :], xts(kt),
                                         start=(kt == 0), stop=(kt == KT - 1))
                    vc = work.tile([HD, SCH], bf16, tag="vc", bufs=2,
                                   name=f"vc{sc}")
                    nc.vector.tensor_copy(vc[:], ps_v[:])
                    vcs["vc"] = vc

                def g_vt(sc=sc, vcs=vcs):
                    vc = vcs["vc"]
                    for j in range(NQS):
                        kb = sc * NQS + j
                        ps_vt = psp.tile([P, P], bf16, tag="t", bufs=2,
                                         name=f"psvt{sc}_{j}")
                        nc.tensor.transpose(
                            ps_vt[:], vc[:, j * P:(j + 1) * P], ident[:])
                        nc.vector.tensor_copy(vp_all[:, kb, 0:HD], ps_vt[:])

                def mk_q(h):
                    def g_q(sc=sc, s0=s0, h=h, xts=xts, st=st):
                        ps_q = psp.tile([P, SCH], f32, tag="a", bufs=3,
                                        name=f"psq{sc}_{h}")
                        for kt in range(KT):
                            nc.tensor.matmul(ps_q[:], wq_sb[:, h, kt, :],
                                             xts(kt),
                                             start=(kt == 0),
                                             stop=(kt == KT - 1))
                        qt = qtsp.tile([HD, SCH], bf16, tag="qts",
                                       name=f"qt{sc}_{h}")
                        rope_evict(ps_q, qt[:], s0)
                        st["qts"].append(qt)
                    return g_q

                return st, [g_k, g_v, g_vt] + [mk_q(h) for h in range(HPC)]

            projq = []

            def take_proj(n):
                k = min(n, len(projq))
                for fn in projq[:k]:
                    fn()
                del projq[:k]

            # chunk 0's projections run inline up front
            st_cur, cl0 = make_proj_work(0, xtg_cur)
            for fn in cl0:
                fn()

            # ================= chunk loop (attention only; next chunk's
            # projections and pending Wo interleave as filler)
            for sc in range(NSC):
                s0 = sc * SCH
                take_proj(len(projq))          # safety: finish proj(sc)
                qts = st_cur["qts"]
                if sc + 1 < NSC:
                    st_next, cls = make_proj_work(sc + 1, issue_x(sc + 1))
                    projq.extend(cls)

                # attention; stage ctxT per head-pair, flush + AllGather.
                # ctx groups are interleaved right behind their diagonal
                # score tile, with Wo filler pacing the scalar-engine exps.
                nkb = NQS * sc + NQS  # key tiles visible to this chunk
                stage = None
                for h in range(HPC):
                    wos.release((sc, h))
                    if h % 2 == 0:
                        stage = stgp.tile([P, 2, SCH], bf16, tag="stg")
                    pts = []
                    ctxns = []

                    def ctx_group(qh, pts=pts, ctxns=ctxns):
                        iqc = NQS * sc + qh
                        ps_c = psp.tile([P, SCH], f32, tag="a", bufs=3,
                                        name=f"psc{sc}_{qh}")
                        for kb in range(iqc + 1):
                            nc.tensor.matmul(
                                ps_c[:, 0:HD + 1],
                                pts[kb][:, qh * P:(qh + 1) * P],
                                vp_all[:, kb, :],
                                start=(kb == 0), stop=(kb == iqc))
                        rden = work.tile([P, 1], f32, tag="rden", bufs=4)
                        nc.vector.reciprocal(rden[:], ps_c[:, HD:HD + 1])
                        ctxn = work.tile([P, HD], bf16, tag="ctxn", bufs=4)
                        nc.vector.tensor_scalar_mul(ctxn[:], ps_c[:, 0:HD],
                                                    rden[:])
                        ctxns.append(ctxn)

                    for kb in range(nkb):
                        diag = kb - NQS * sc
                        off = diag * P if diag > 0 else 0
                        ps_s = psp.tile([P, SCH], f32, tag="a", bufs=3)
                        nc.tensor.matmul(ps_s[:, off:],
                                         kT_all[:, kb * P:(kb + 1) * P],
                                         qts[h][:, off:], start=True, stop=True)
                        # fast vector evict frees the PSUM bank at PE pace;
                        # the slow exp then runs decoupled, from SBUF
                        sstg = work.tile([P, SCH], bf16, tag="sstg", bufs=4)
                        nc.vector.tensor_copy(sstg[:, off:], ps_s[:, off:])
                        pt = ptp.tile([P, SCH], bf16, tag="pt")
                        nc.scalar.activation(out=pt[:, off:], in_=sstg[:, off:],
                                             func=Exp, scale=SCALE)
                        if 0 <= diag < NQS:
                            nc.vector.tensor_mul(
                                pt[:, diag * P:(diag + 1) * P],
                                pt[:, diag * P:(diag + 1) * P], trim[:])
                        pts.append(pt)
                        wos.take(1)
                        if sc == 0 and kb % 2 == 1:
                            take_proj(1)
                        if diag >= 0:
                            ctx_group(diag)

                    for qh in range(NQS):
                        ps_t = psp.tile([P, P], bf16, tag="t", bufs=2)
                        nc.tensor.transpose(ps_t[:], ctxns[qh][:], ident[:])
                        nc.vector.tensor_copy(
                            stage[:, h % 2, qh * P:(qh + 1) * P], ps_t[:])
                    wos.take(2)

                    if h % 2 == 1:
                        pr = h // 2
                        nc.scalar.dma_start(
                            out=cc_ins[sc][pr][:].rearrange(
                                "(h p) c -> p h c", p=P),
                            in_=stage[:])
                        nc.gpsimd.collective_compute(
                            "AllGather",
                            mybir.AluOpType.bypass,
                            replica_groups=[list(range(NC))],
                            ins=[cc_ins[sc][pr][:]],
                            outs=[cc_outs[sc][pr][:]],
                        )
                        wos.take(2)

                    # next chunk's projections as end-of-slot filler
                    take_proj(2)

                # finish all Wo work whose AllGathers are long done, then
                # queue this chunk's split (consumable from next chunk)
                wos.drain()
                if sc < NSC - 1:
                    queue_wo_split(sc)
                    st_cur = st_next

            # ================= tail: last split, single pass over 4 ob
            # accumulators (scores/ctx PSUM banks are free now), kt ordered
            # by head so tiles are consumed as their AllGathers land
            spl = NSC - 1
            accs3 = []
            for ob in range(4):
                tg = "o" if ob < 2 else "a"
                a = psp.tile([P, SCH], f32, tag=tg, bufs=(2 if tg == "o" else 3),
                             name=f"wo3_{ob}")
                accs3.append(a)
            kts3 = sorted(range(KT), key=kt_loc)
            PF3 = 6
            ccts3 = {}

            def issue3(i):
                pr, t = kt_loc(kts3[i])
                cct = ccp.tile([P, SCH], bf16, tag="ccp", name=f"cc3_{i}")
                nc.sync.dma_start(
                    out=cct[:],
                    in_=cc_outs[spl][pr][:].rearrange("(t p) s -> p t s",
                                                      p=P)[:, t, :])
                ccts3[i] = cct

            for j in range(PF3):
                issue3(j)
            for i, kt in enumerate(kts3):
                if i + PF3 < len(kts3):
                    issue3(i + PF3)
                cct = ccts3.pop(i)
                for ob in range(4):
                    nc.tensor.matmul(
                        accs3[ob][:], wo_sb[:, kt, ob * P:(ob + 1) * P],
                        cct[:], start=(i == 0), stop=(i == len(kts3) - 1))
            for ob in range(4):
                o_sb = osbp.tile([P, SCH], bf16, tag="osb", name=f"osb3_{ob}")
                nc.vector.tensor_copy(o_sb[:], accs3[ob][:])
                nc.scalar.dma_start(
                    out=out[ob * P:(ob + 1) * P, spl * SCH:(spl + 1) * SCH],
                    in_=o_sb[:])

    return nc


_PROGRAM_CACHE = {}


def _get_program():
    if "nc" not in _PROGRAM_CACHE:
        nc = _build_program()
        _split_multi_waits(nc, maxw=1)
        _PROGRAM_CACHE["nc"] = nc
    return _PROGRAM_CACHE["nc"]


def _rope_tables_T():
    inv_freq = (1.0 / (10000.0 ** (np.arange(0, HD, 2, dtype=np.float32) / HD))
                ).astype(np.float32)
    ang = np.arange(S, dtype=np.float32)[:, None] * inv_freq[None, :]
    ang = np.concatenate([ang, ang], axis=-1)  # [S, HD]
    cosT = np.ascontiguousarray(np.cos(ang).T).astype(ml_dtypes.bfloat16)
    sinT = np.ascontiguousarray(np.sin(ang).T).astype(ml_dtypes.bfloat16)
    return cosT, sinT


def _prep_in_maps(x, Wq, Wk, Wv, Wo):
    bf = ml_dtypes.bfloat16
    x2d = np.asarray(x, np.float32).reshape(S, D).astype(bf)
    # [D, S] -> [P, NSC, KT, SCH]
    xT_dev = np.ascontiguousarray(
        x2d.T.reshape(KT, P, NSC, SCH).transpose(1, 2, 0, 3))
    cosT, sinT = _rope_tables_T()
    tri_np = (np.arange(P)[:, None] <= np.arange(P)[None, :]).astype(bf)

    def wtiles(Wslice, width):
        return np.ascontiguousarray(
            np.asarray(Wslice, np.float32).astype(bf)
            .reshape(KT, P, width).transpose(1, 0, 2))

    in_maps = []
    for c in range(NC):
        wq_c = np.asarray(Wq[:, c * OC:(c + 1) * OC], np.float32).astype(bf)
        wq_t = np.ascontiguousarray(
            wq_c.reshape(KT, P, HPC, HD).transpose(1, 2, 0, 3))
        in_maps.append({
            "xT": xT_dev,
            "wq": wq_t,
            "wk": wtiles(Wk[:, c * HD:(c + 1) * HD], HD),
            "wv": wtiles(Wv[:, c * HD:(c + 1) * HD], HD),
            "wo": wtiles(Wo[:, c * OC:(c + 1) * OC], OC),
            "cosT": cosT,
            "sinT": sinT,
            "tri": tri_np,
        })
    return in_maps


def _run(inputs, trace=False):
    nc = _get_program()
    in_maps = _prep_in_maps(inputs["x"], inputs["Wq"], inputs["Wk"],
                            inputs["Wv"], inputs["Wo"])
    res = run_bass_kernel_spmd(nc, in_maps, core_ids=list(range(NC)),
                               trace=trace)
    out = np.concatenate(
        [np.asarray(res.results[c]["out"]).astype(np.float32).T
         for c in range(NC)], axis=1)
    return out.reshape(B, S, D).astype(np.float32), res


def kernel(**inputs):
    out, _ = _run(inputs, trace=False)
    return out


# revision 37
# speedup vs baseline: 1.0974x; 1.0974x over previous
"""GQA (B=1, S=2048, D=4096, H=32, G=8) on 8 TRN2 NeuronCores.

Tensor-parallel over heads: core c owns query heads 4c..4c+3 and KV group c.
v2: all-bf16 datapath (weights/x/k/q/probs/ctx/Wo and the AllGather wire),
SCH=512 sequence chunks, per-head-pair AllGathers issued mid-chunk, Wo
matmuls interleaved into the attention chunks as tensor-engine filler, and
output produced transposed ([OC, S]) so Wo keeps its weights stationary.
Host slices weights / transposes x / re-transposes the output.

Self-contained: no sibling imports; hardcoded shapes.
"""
import contextlib
import ctypes
import math
import os
import sys
import types

import ml_dtypes
import numpy as np

os.environ.setdefault("MYCRO_LOCAL_CACHE", "1")

for _p in ("/opt/trn_rl_repo", "/root/.axon_site/_ro/trn_rl_repo"):
    if _p not in sys.path and os.path.isdir(_p):
        sys.path.append(_p)

import concourse.bass as bass
import concourse.tile as tile
from concourse import mybir
from concourse.bass_utils import run_bass_kernel_spmd
from concourse.masks import make_identity

# ---------------------------------------------------------------- profiling shim
_SO_PATH = "/opt/axon/libaxon_pjrt.so"
_hook_holder = [None]


def _ntff_profile_via_ctypes(so_path):
    try:
        lib = ctypes.CDLL(so_path)
    except OSError:
        return None
    if not hasattr(lib, "axon_start_nrt_profile"):
        return None
    lib.axon_start_nrt_profile.argtypes = [
        ctypes.POINTER(ctypes.c_int64),
        ctypes.c_size_t,
    ]
    lib.axon_start_nrt_profile.restype = ctypes.c_int64
    lib.axon_stop_nrt_profile.argtypes = [ctypes.c_char_p]
    lib.axon_stop_nrt_profile.restype = ctypes.c_int64

    @contextlib.contextmanager
    def _hook(output_dir, device_ids):
        import jax

        jax.devices()
        if device_ids:
            ids = (ctypes.c_int64 * len(device_ids))(*device_ids)
            rc = lib.axon_start_nrt_profile(ids, len(device_ids))
        else:
            rc = lib.axon_start_nrt_profile(None, 0)
        if rc != 0:
            raise RuntimeError(f"axon_start_nrt_profile rc={rc}")
        try:
            yield
        finally:
            n = lib.axon_stop_nrt_profile(str(output_dir).encode())
            if n <= 0:
                print(f"WARNING: ntff capture wrote {n} files", file=sys.stderr)

    return _hook


def _install_prof_shim():
    if "antenv.axon_hooks" not in sys.modules:
        mod = types.ModuleType("antenv.axon_hooks")
        mod.set_axon_ntff_profile_hook = lambda h: _hook_holder.__setitem__(0, h)
        mod.get_axon_ntff_profile_hook = lambda: _hook_holder[0]
        sys.modules["antenv.axon_hooks"] = mod
    _hook_holder[0] = _ntff_profile_via_ctypes(_SO_PATH)
    import concourse.bass_utils as bu

    bu.upload_artifacts = lambda tmpdir: tmpdir


_install_prof_shim()

# ------------------------------------------------------------- wait-split pass
def _split_multi_waits(nc, maxw=1):
    """walrus in this container allows only one sync-wait per instruction;
    split extras onto nops inserted before the offender (same engine/block)."""

    def _remove_by_name(name):
        for f in nc.m.functions:
            for bb in f.blocks:
                for i, inst in enumerate(bb.instructions):
                    if inst.name == name:
                        lst = bb.instructions
                        del lst[i]
                        bb.instructions = lst
                        return inst
        raise KeyError(name)

    offenders = []
    for f in nc.m.functions:
        for bb in f.blocks:
            for inst in bb.instructions:
                si = inst.sync_info
                if si and si.on_wait and len(si.on_wait) > maxw:
                    offenders.append(inst.name)
    for name in offenders:
        target = None
        for f in nc.m.functions:
            for bb in f.blocks:
                for idx, inst in enumerate(bb.instructions):
                    if inst.name == name:
                        target = (bb, inst)
                        break
                if target:
                    break
            if target:
                break
        bb, inst = target
        waits = list(inst.sync_info.on_wait)
        updates = list(inst.sync_info.on_update or [])
        chunks = [waits[i:i + maxw] for i in range(0, len(waits), maxw)]
        nops = []
        for ch in chunks[:-1]:
            bnop = nc.engines[inst.engine].nop(nofuse=True, hint="waitsplit")
            nop_inst = _remove_by_name(bnop.ins.name)
            nop_inst.sync_info = mybir.SyncInfo(on_wait=ch, on_update=[])
            nops.append(nop_inst)
        inst.sync_info = mybir.SyncInfo(on_wait=chunks[-1], on_update=updates)
        lst = bb.instructions
        idx = next(i for i, x in enumerate(lst) if x.name == name)
        lst[idx:idx] = nops
        bb.instructions = lst
    return len(offenders)


# ------------------------------------------------------------------- constants
B, S, D = 1, 2048, 4096
H, G = 32, 8
HD = D // H            # 128
NC = 8                 # cores
HPC = H // NC          # q heads per core = 4
OC = D // NC           # out columns per core = 512
P = 128
KT = D // P            # 32 contraction tiles over D
SCH = 512              # sequence chunk width
NSC = S // SCH         # 4
NKB = S // P           # 16 key tiles of 128
NQS = SCH // P         # 4 q-subchunks per chunk
SCALE = float(1.0 / math.sqrt(float(HD)))

f32 = mybir.dt.float32
bf16 = mybir.dt.bfloat16

Copy = mybir.ActivationFunctionType.Copy
Exp = mybir.ActivationFunctionType.Exp


class _WoSched:
    """FIFO of deferred Wo closures with release markers: a ("wait", key)
    entry blocks consumption until release(key) — used to hold units back
    until their AllGather has had time to complete."""

    def __init__(self):
        self.units = []
        self.released = set()

    def add(self, fn):
        self.units.append(fn)

    def add_marker(self, key):
        self.units.append(("wait", key))

    def release(self, key):
        self.released.add(key)

    def take(self, n):
        while n > 0 and self.units:
            head = self.units[0]
            if isinstance(head, tuple):
                if head[1] in self.released:
                    self.units.pop(0)
                    continue
                return
            self.units.pop(0)()
            n -= 1

    def drain(self):
        self.take(len(self.units) + 8)

    def force_drain(self):
        for u in self.units:
            if not isinstance(u, tuple):
                u()
        self.units = []


def _build_program():
    nc = bass.Bass()
    xT = nc.declare_dram_parameter("xT", [P, NSC, KT, SCH], bf16, isOutput=False)
    wq = nc.declare_dram_parameter("wq", [P, HPC, KT, HD], bf16, isOutput=False)
    wk = nc.declare_dram_parameter("wk", [P, KT, HD], bf16, isOutput=False)
    wv = nc.declare_dram_parameter("wv", [P, KT, HD], bf16, isOutput=False)
    wo = nc.declare_dram_parameter("wo", [P, KT, OC], bf16, isOutput=False)
    cosT = nc.declare_dram_parameter("cosT", [HD, S], bf16, isOutput=False)
    sinT = nc.declare_dram_parameter("sinT", [HD, S], bf16, isOutput=False)
    tri = nc.declare_dram_parameter("tri", [P, P], bf16, isOutput=False)
    out = nc.declare_dram_parameter("out", [OC, S], bf16, isOutput=True)

    # one AllGather per (chunk, head-pair): in [2*P, SCH], out [NC*2*P, SCH]
    cc_ins = [[nc.dram_tensor(f"cc_in{sc}_{pr}", [2 * P, SCH], bf16)
               for pr in range(2)] for sc in range(NSC)]
    cc_outs = [[nc.dram_tensor(f"cc_out{sc}_{pr}", [NC * 2 * P, SCH], bf16,
                               addr_space="Shared")
                for pr in range(2)] for sc in range(NSC)]
    bar_in = nc.dram_tensor("bar_in", [P, 8], bf16)
    bar_out = nc.dram_tensor("bar_out", [NC * P, 8], bf16,
                             addr_space="Shared")

    # global 128-row D-block kt = 4*rank + head -> (pair, tile within cc_out)
    def kt_loc(kt):
        r, h = kt // HPC, kt % HPC
        return h // 2, r * 2 + (h % 2)

    with tile.TileContext(nc) as tc:
        with (
            tc.tile_pool(name="singles", bufs=1) as singles,
            tc.tile_pool(name="xp", bufs=6) as xpp,
            tc.tile_pool(name="qts", bufs=6) as qtsp,
            tc.tile_pool(name="pt", bufs=20) as ptp,
            tc.tile_pool(name="work", bufs=4) as work,
            tc.tile_pool(name="stg", bufs=3) as stgp,
            tc.tile_pool(name="ccp", bufs=12) as ccp,
            tc.tile_pool(name="osb", bufs=3) as osbp,
            tc.tile_pool(name="ps", bufs=1, space="PSUM") as psp,
        ):
            # ---- constants loaded in compute-dependency order
            # rank-alignment barrier: fires immediately on the CC ring so the
            # startup skew is absorbed while weights stream in
            nc.gpsimd.collective_compute(
                "AllGather", mybir.AluOpType.bypass,
                replica_groups=[list(range(NC))],
                ins=[bar_in[:]], outs=[bar_out[:]])

            identf = singles.tile([P, P], f32)
            make_identity(nc, identf[:])
            ident = singles.tile([P, P], bf16)
            nc.vector.tensor_copy(ident[:], identf[:])
            trim = singles.tile([P, P], bf16)
            nc.sync.dma_start(out=trim[:], in_=tri[:])

            def issue_x(sc):
                tiles = []
                for g in range(KT // 8):
                    t = xpp.tile([P, 8, SCH], bf16, tag="xp", bufs=6)
                    nc.sync.dma_start(out=t[:], in_=xT[:, sc, 8 * g:8 * g + 8, :])
                    tiles.append(t)
                return tiles

            # stagger wk pieces with x pieces so the first projections can
            # begin as soon as ~0.5 MB has landed
            wk_sb = singles.tile([P, KT, HD], bf16)
            wv_sb = singles.tile([P, KT, HD], bf16)
            xtg_cur = []
            for g in range(KT // 8):
                nc.sync.dma_start(out=wk_sb[:, 8 * g:8 * g + 8, :],
                                  in_=wk[:, 8 * g:8 * g + 8, :])
                t = xpp.tile([P, 8, SCH], bf16, tag="xp", bufs=6)
                nc.sync.dma_start(out=t[:], in_=xT[:, 0, 8 * g:8 * g + 8, :])
                xtg_cur.append(t)
            nc.sync.dma_start(out=wv_sb[:], in_=wv[:])

            wq_sb = singles.tile([P, HPC, KT, HD], bf16)
            nc.sync.dma_start(out=wq_sb[:, 0], in_=wq[:, 0])
            cos_sb = singles.tile([HD, S], bf16)
            nc.sync.dma_start(out=cos_sb[:], in_=cosT[:])
            sin_sb = singles.tile([HD, S], bf16)
            nc.sync.dma_start(out=sin_sb[:], in_=sinT[:])
            for h in range(1, HPC):
                nc.sync.dma_start(out=wq_sb[:, h], in_=wq[:, h])
            wo_sb = singles.tile([P, KT, OC], bf16)
            nc.sync.dma_start(out=wo_sb[:], in_=wo[:])

            kT_all = singles.tile([HD, S], bf16)
            vp_all = singles.tile([P, NKB, HD + 1], bf16)
            nc.vector.memset(vp_all[:], 1.0)

            wos = _WoSched()

            # ---------- Wo split `spl`: two ob-pair passes over all 32 kt
            def queue_wo_split(spl):
                kts = sorted(range(KT), key=lambda kt: kt_loc(kt))
                wos.add_marker((spl + 1, 3 if spl == 0 else 0))
                for pss in (0, 1):
                    obs = (2 * pss, 2 * pss + 1)
                    accs = {}
                    ccts = {}
                    PF = 6

                    def issue_dma(i, kts=kts, ccts=ccts):
                        pr, t = kt_loc(kts[i])
                        cct = ccp.tile([P, SCH], bf16, tag="ccp")
                        nc.sync.dma_start(
                            out=cct[:],
                            in_=cc_outs[spl][pr][:].rearrange(
                                "(t p) s -> p t s", p=P)[:, t, :])
                        ccts[i] = cct

                    def emit(i, pss=pss, obs=obs, kts=kts, accs=accs,
                             ccts=ccts, PF=PF, issue_dma=issue_dma):
                        if i == 0:
                            for ob in obs:
                                accs[ob] = psp.tile([P, SCH], f32, tag="o",
                                                    bufs=2, name=f"wo{spl}_{ob}")
                            for j in range(min(PF, len(kts))):
                                issue_dma(j)
                        if i + PF < len(kts):
                            issue_dma(i + PF)
                        cct = ccts.pop(i)
                        first, last = i == 0, i == len(kts) - 1
                        for ob in obs:
                            nc.tensor.matmul(
                                accs[ob][:], wo_sb[:, kts[i], ob * P:(ob + 1) * P],
                                cct[:], start=first, stop=last)
                        if last:
                            for ob in obs:
                                ps_o = accs.pop(ob)
                                o_sb = osbp.tile([P, SCH], bf16, tag="osb")
                                nc.vector.tensor_copy(o_sb[:], ps_o[:])
                                nc.scalar.dma_start(
                                    out=out[ob * P:(ob + 1) * P,
                                            spl * SCH:(spl + 1) * SCH],
                                    in_=o_sb[:])

                    for i in range(len(kts)):
                        if pss == 0 and i > 0 and kt_loc(kts[i])[0] == 1 \
                                and kt_loc(kts[i - 1])[0] == 0:
                            wos.add_marker((spl + 1, 3))
                        wos.add(lambda i=i, emit=emit: emit(i))

            def rope_evict(ps_t, dst, tab0):
                """ps_t: PSUM [HD, SCH] pre-rope; writes dst [HD, SCH] bf16
                using rope tables at absolute position tab0."""
                rot = work.tile([HD, SCH], f32, tag="rot", bufs=2)
                nc.vector.tensor_scalar_mul(rot[0:64, :], ps_t[64:128, :], -1.0)
                nc.vector.tensor_copy(rot[64:128, :], ps_t[0:64, :])
                m1 = work.tile([HD, SCH], f32, tag="m1", bufs=2)
                nc.vector.tensor_mul(m1[:], ps_t[:], cos_sb[:, tab0:tab0 + SCH])
                nc.vector.tensor_mul(rot[:], rot[:], sin_sb[:, tab0:tab0 + SCH])
                nc.vector.tensor_add(dst, m1[:], rot[:])

            # ---------- projection work for chunk sc as filler closures;
            # dispatched inside the PREVIOUS chunk's attention
            def make_proj_work(sc, xtg):
                s0 = sc * SCH
                st = {"qts": []}

                def xts(kt, xtg=xtg):
                    return xtg[kt // 8][:, kt % 8, :]

                vcs = {}

                def g_k(sc=sc, s0=s0, xts=xts):
                    ps_k = psp.tile([P, SCH], f32, tag="a", bufs=4,
                                    name=f"psk{sc}")
                    for kt in range(KT):
                        nc.tensor.matmul(ps_k[:], wk_sb[:, kt, :], xts(kt),
                                         start=(kt == 0), stop=(kt == KT - 1))
                    rope_evict(ps_k, kT_all[:, s0:s0 + SCH], s0)

                def g_v(sc=sc, xts=xts, vcs=vcs):
                    ps_v = psp.tile([P, SCH], f32, tag="a", bufs=4,
                                    name=f"psv{sc}")
                    for kt in range(KT):
                        nc.tensor.matmul(ps_v[:], wv_sb[:, kt, :], xts(kt),
                                         start=(kt == 0), stop=(kt == KT - 1))
                    vc = work.tile([HD, SCH], bf16, tag="vc", bufs=2,
                                   name=f"vc{sc}")
                    nc.vector.tensor_copy(vc[:], ps_v[:])
                    vcs["vc"] = vc

                def g_vt(sc=sc, vcs=vcs):
                    vc = vcs["vc"]
                    for j in range(NQS):
                        kb = sc * NQS + j
                        ps_vt = psp.tile([P, P], bf16, tag="t", bufs=2,
                                         name=f"psvt{sc}_{j}")
                        nc.tensor.transpose(
                            ps_vt[:], vc[:, j * P:(j + 1) * P], ident[:])
                        nc.vector.tensor_copy(vp_all[:, kb, 0:HD], ps_vt[:])

                def mk_q(h):
                    def g_q(sc=sc, s0=s0, h=h, xts=xts, st=st):
                        ps_q = psp.tile([P, SCH], f32, tag="a", bufs=4,
                                        name=f"psq{sc}_{h}")
                        for kt in range(KT):
                            nc.tensor.matmul(ps_q[:], wq_sb[:, h, kt, :],
                                             xts(kt),
                                             start=(kt == 0),
                                             stop=(kt == KT - 1))
                        qt = qtsp.tile([HD, SCH], bf16, tag="qts",
                                       name=f"qt{sc}_{h}")
                        rope_evict(ps_q, qt[:], s0)
                        st["qts"].append(qt)
                    return g_q

                return st, [g_k, g_v, g_vt] + [mk_q(h) for h in range(HPC)]

            projq = []

            def take_proj(n):
                k = min(n, len(projq))
                for fn in projq[:k]:
                    fn()
                del projq[:k]

            # chunk 0's projections run inline up front
            st_cur, cl0 = make_proj_work(0, xtg_cur)
            for fn in cl0:
                fn()

            # ================= chunk loop (attention only; next chunk's
            # projections and pending Wo interleave as filler)
            for sc in range(NSC):
                s0 = sc * SCH
                take_proj(len(projq))          # safety: finish proj(sc)
                qts = st_cur["qts"]
                if sc + 1 < NSC:
                    st_next, cls = make_proj_work(sc + 1, issue_x(sc + 1))
                    projq.extend(cls)

                # attention; stage ctxT per head-pair, flush + AllGather.
                # ctx groups are interleaved right behind their diagonal
                # score tile, with Wo filler pacing the scalar-engine exps.
                nkb = NQS * sc + NQS  # key tiles visible to this chunk
                stage = None
                for h in range(HPC):
                    wos.release((sc, h))
                    if h % 2 == 0:
                        stage = stgp.tile([P, 2, SCH], bf16, tag="stg")
                    pts = []
                    ctxns = []

                    def ctx_group(qh, pts=pts, ctxns=ctxns):
                        iqc = NQS * sc + qh
                        ps_c = psp.tile([P, SCH], f32, tag="a", bufs=4,
                                        name=f"psc{sc}_{qh}")
                        for kb in range(iqc + 1):
                            nc.tensor.matmul(
                                ps_c[:, 0:HD + 1],
                                pts[kb][:, qh * P:(qh + 1) * P],
                                vp_all[:, kb, :],
                                start=(kb == 0), stop=(kb == iqc))
                        rden = work.tile([P, 1], f32, tag="rden", bufs=4)
                        nc.vector.reciprocal(rden[:], ps_c[:, HD:HD + 1])
                        ctxn = work.tile([P, HD], bf16, tag="ctxn", bufs=4)
                        nc.vector.tensor_scalar_mul(ctxn[:], ps_c[:, 0:HD],
                                                    rden[:])
                        ctxns.append(ctxn)

                    for kb in range(nkb):
                        diag = kb - NQS * sc
                        off = diag * P if diag > 0 else 0
                        ps_s = psp.tile([P, SCH], f32, tag="a", bufs=4)
                        nc.tensor.matmul(ps_s[:, off:],
                                         kT_all[:, kb * P:(kb + 1) * P],
                                         qts[h][:, off:], start=True, stop=True)
                        pt = ptp.tile([P, SCH], bf16, tag="pt")
                        nc.scalar.activation(out=pt[:, off:], in_=ps_s[:, off:],
                                             func=Exp, scale=SCALE)
                        if 0 <= diag < NQS:
                            nc.vector.tensor_mul(
                                pt[:, diag * P:(diag + 1) * P],
                                pt[:, diag * P:(diag + 1) * P], trim[:])
                        pts.append(pt)
                        wos.take(1)
                        if sc == 0 and kb % 2 == 1:
                            take_proj(1)
                        if diag >= 0:
                            ctx_group(diag)

                    for qh in range(NQS):
                        ps_t = psp.tile([P, P], bf16, tag="t", bufs=2)
                        nc.tensor.transpose(ps_t[:], ctxns[qh][:], ident[:])
                        nc.vector.tensor_copy(
                            stage[:, h % 2, qh * P:(qh + 1) * P], ps_t[:])
                    wos.take(2)

                    if h % 2 == 1:
                        pr = h // 2
                        nc.scalar.dma_start(
                            out=cc_ins[sc][pr][:].rearrange(
                                "(h p) c -> p h c", p=P),
                            in_=stage[:])
                        nc.gpsimd.collective_compute(
                            "AllGather",
                            mybir.AluOpType.bypass,
                            replica_groups=[list(range(NC))],
                            ins=[cc_ins[sc][pr][:]],
                            outs=[cc_outs[sc][pr][:]],
                        )
                        wos.take(2)

                    # next chunk's projections as end-of-slot filler
                    take_proj(2)

                # finish all Wo work whose AllGathers are long done, then
                # queue this chunk's split (consumable from next chunk)
                wos.drain()
                if sc < NSC - 1:
                    queue_wo_split(sc)
                    st_cur = st_next

            # ================= tail: last split, single pass over 4 ob
            # accumulators (scores/ctx PSUM banks are free now), kt ordered
            # by head so tiles are consumed as their AllGathers land
            spl = NSC - 1
            accs3 = []
            for ob in range(4):
                tg = "o" if ob < 2 else "a"
                a = psp.tile([P, SCH], f32, tag=tg, bufs=(2 if tg == "o" else 4),
                             name=f"wo3_{ob}")
                accs3.append(a)
            kts3 = sorted(range(KT), key=kt_loc)
            PF3 = 6
            ccts3 = {}

            def issue3(i):
                pr, t = kt_loc(kts3[i])
                cct = ccp.tile([P, SCH], bf16, tag="ccp", name=f"cc3_{i}")
                nc.sync.dma_start(
                    out=cct[:],
                    in_=cc_outs[spl][pr][:].rearrange("(t p) s -> p t s",
                                                      p=P)[:, t, :])
                ccts3[i] = cct

            for j in range(PF3):
                issue3(j)
            for i, kt in enumerate(kts3):
                if i + PF3 < len(kts3):
                    issue3(i + PF3)
                cct = ccts3.pop(i)
                for ob in range(4):
                    nc.tensor.matmul(
                        accs3[ob][:], wo_sb[:, kt, ob * P:(ob + 1) * P],
                        cct[:], start=(i == 0), stop=(i == len(kts3) - 1))
            for ob in range(4):
                o_sb = osbp.tile([P, SCH], bf16, tag="osb", name=f"osb3_{ob}")
                nc.vector.tensor_copy(o_sb[:], accs3[ob][:])
                nc.scalar.dma_start(
                    out=out[ob * P:(ob + 1) * P, spl * SCH:(spl + 1) * SCH],
                    in_=o_sb[:])

    return nc


_PROGRAM_CACHE = {}


def _get_program():
    if "nc" not in _PROGRAM_CACHE:
        nc = _build_program()
        _split_multi_waits(nc, maxw=1)
        _PROGRAM_CACHE["nc"] = nc
    return _PROGRAM_CACHE["nc"]


def _rope_tables_T():
    inv_freq = (1.0 / (10000.0 ** (np.arange(0, HD, 2, dtype=np.float32) / HD))
                ).astype(np.float32)
    ang = np.arange(S, dtype=np.float32)[:, None] * inv_freq[None, :]
    ang = np.concatenate([ang, ang], axis=-1)  # [S, HD]
    cosT = np.ascontiguousarray(np.cos(ang).T).astype(ml_dtypes.bfloat16)
    sinT = np.ascontiguousarray(np.sin(ang).T).astype(ml_dtypes.bfloat16)
    return cosT, sinT


def _prep_in_maps(x, Wq, Wk, Wv, Wo):
    bf = ml_dtypes.bfloat16
    x2d = np.asarray(x, np.float32).reshape(S, D).astype(bf)
    # [D, S] -> [P, NSC, KT, SCH]
    xT_dev = np.ascontiguousarray(
        x2d.T.reshape(KT, P, NSC, SCH).transpose(1, 2, 0, 3))
    cosT, sinT = _rope_tables_T()
    tri_np = (np.arange(P)[:, None] <= np.arange(P)[None, :]).astype(bf)

    def wtiles(Wslice, width):
        return np.ascontiguousarray(
            np.asarray(Wslice, np.float32).astype(bf)
            .reshape(KT, P, width).transpose(1, 0, 2))

    in_maps = []
    for c in range(NC):
        wq_c = np.asarray(Wq[:, c * OC:(c + 1) * OC], np.float32).astype(bf)
        wq_t = np.ascontiguousarray(
            wq_c.reshape(KT, P, HPC, HD).transpose(1, 2, 0, 3))
        in_maps.append({
            "xT": xT_dev,
            "wq": wq_t,
            "wk": wtiles(Wk[:, c * HD:(c + 1) * HD], HD),
            "wv": wtiles(Wv[:, c * HD:(c + 1) * HD], HD),
            "wo": wtiles(Wo[:, c * OC:(c + 1) * OC], OC),
            "cosT": cosT,
            "sinT": sinT,
            "tri": tri_np,
        })
    return in_maps


def _run(inputs, trace=False):
    nc = _get_program()
    in_maps = _prep_in_maps(inputs["x"], inputs["Wq"], inputs["Wk"],
                            inputs["Wv"], inputs["Wo"])
    res = run_bass_kernel_spmd(nc, in_maps, core_ids=list(range(NC)),
                               trace=trace)
    out = np.concatenate(
        [np.asarray(res.results[c]["out"]).astype(np.float32).T
         for c in range(NC)], axis=1)
    return out.reshape(B, S, D).astype(np.float32), res


def kernel(**inputs):
    out, _ = _run(inputs, trace=False)
    return out
